# revision 21
# baseline (speedup 1.0000x reference)
"""GCGRUCell (SplineConv-based GRU cell) Trainium2 kernel.

Strategy (8 NeuronCores, SPMD):
- Nodes are partitioned contiguously across cores (750/core, padded to 768);
  each core owns all edges whose destination lies in its node range
  (host groups edges by destination; source features are replicated so no
  halo exchange is needed).
- The six SplineConvs share one sparse structure: for each edge, the
  degree-1 B-spline basis over K=5^3 kernels factorizes as
  kron(v1, v2, v3) with v_d[j] = relu(1 - |j - u_d|) (5-wide hat weights).
  Per destination node n we form T_n[k, i] = sum_{e->n} basis[e,k] *
  xh[src_e, i] with xh = [x | hidden] (96 features). Each 128-row chunk
  holds two nodes' edge rows (64 each, partitions 0:64 / 64:128); two
  64-deep matmuls per chunk write T_A / T_B into disjoint PSUM columns,
  so the gathered-feature tile stays fully contiguous (no block-diagonal
  zeros, large DMA bursts).
- All six convs then reduce to dense matmuls over T: out = sum_i
  T[:, i, :]^T @ W[., i, .] accumulated in PSUM, plus a root/bias matmul
  (contraction rows = raw x/h features + a ones row). 1/deg mean
  normalization is folded into v1 on the host.
- PSUM->SBUF T drains rotate across DVE/ACT/Pool so no single engine
  throttles the PE stream; GRU combine runs on ACT+DVE per node block.
"""

import os
import numpy as np

# front matmul operands (basis matrix A and gathered features) in fp8e4m3:
# halves the two dominant DMA streams. Both operands must share the dtype --
# mixed fp8 x fp16 matmuls kill the exec unit (NRT_EXEC_UNIT_UNRECOVERABLE).
F8 = os.environ.get("KERN_F8", "") != ""

# ---------------- problem constants (hardcoded per contract) ----------------
N, E, CIN, CHID, DIM, KS = 6000, 192000, 32, 64, 3, 5
K = KS ** DIM
NCORES = 8
NPN = N // NCORES          # 750 real nodes per core
BN = 128                   # nodes per block
NBLK = 6                   # blocks per core (768 padded nodes)
NPC = BN * NBLK            # 768 padded nodes per core
FEAT = CIN + CHID          # 96
RPN = 64                   # edge rows per node (max supported degree)
ROWS_BLK = BN * RPN        # 8192
CH_BLK = ROWS_BLK // 128   # 64 chunks per block
OC = 3 * CHID              # 192 x-conv output columns (r|z|n)
OH = 2 * CHID              # 128 h-conv output columns (r|z)

_cache = {}


def _patch_tile_drain(tile_mod, mybir):
    """This walrus build rejects >1-2 sync waits on the tail Drain
    ("Too many sync wait commands"); spread waits across nops instead."""
    if getattr(tile_mod.TileContext, "_drain_patched", False):
        return

    def _drain_and_barrier(self, tick_clock, wait_clock):
        drain_inst = self.nc.sync.drain()
        wait_clock.add_sem_waits(
            drain_inst.ins, tile_mod.ScopedClock({None: tick_clock.global_clock})
        )
        si = drain_inst.ins.sync_info
        waits = list(si.on_wait or [])
        if len(waits) > 1:
            si.on_wait = [waits[0]]
            for w in waits[1:]:
                nop = self.nc.sync.nop(nofuse=True)
                nsi = nop.ins.sync_info
                if nsi is None:
                    nop.ins.sync_info = mybir.SyncInfo(on_wait=[w], on_update=[])
                else:
                    nsi.on_wait = [w]
        self.nc.all_engine_barrier()
        assert self.sems is not None
        popped = self.nc._tile_sem_poison_stack.pop()
        assert popped is self._sem_poison
        self.nc.clear_and_free_semaphores(list(self.sems.allocated().values()))
        self.nc.all_engine_barrier()

    tile_mod.TileContext._drain_and_barrier = _drain_and_barrier
    tile_mod.TileContext._drain_patched = True


def _build_program():
    import concourse.bass as bass
    import concourse.tile as tile
    from concourse import mybir, bacc, library_config

    _patch_tile_drain(tile, mybir)

    f16, f32 = mybir.dt.float16, mybir.dt.float32
    f8 = mybir.dt.float8e4
    alu = mybir.AluOpType
    act = mybir.ActivationFunctionType

    nc = bacc.Bacc()
    d_gf = nc.declare_dram_parameter(
        "gf", [64, NBLK * CH_BLK * 2 * FEAT], f8 if F8 else f16, isOutput=False
    )
    d_am = nc.declare_dram_parameter(
        "am", [64, NBLK * CH_BLK * 2 * 128], f8 if F8 else f16, isOutput=False
    )
    d_wx = nc.declare_dram_parameter("wx", [128, CIN * OC], f16, isOutput=False)
    d_wh = nc.declare_dram_parameter("wh", [128, CHID * OH], f16, isOutput=False)
    d_xht = nc.declare_dram_parameter("xht", [FEAT + 1, NPC], f16, isOutput=False)
    d_rx = nc.declare_dram_parameter("rx", [FEAT + 1, OC], f16, isOutput=False)
    d_rh = nc.declare_dram_parameter("rh", [FEAT + 1, OH], f16, isOutput=False)
    d_hid = nc.declare_dram_parameter("hid", [128, NBLK * CHID], f32, isOutput=False)
    d_out = nc.declare_dram_parameter("out", [128, NBLK * CHID], f32, isOutput=True)

    gfv = d_gf[:].rearrange("p (b c n e) -> p b c n e", c=CH_BLK, n=2, e=FEAT)
    amv = d_am[:].rearrange("p (b c n e) -> p b c n e", c=CH_BLK, n=2, e=128)

    with tile.TileContext(nc) as tc:
        with (
            tc.tile_pool(name="const", bufs=1) as cp,
            tc.tile_pool(name="apool", bufs=2) as apool,
            tc.tile_pool(name="gcp", bufs=2) as gcp,
            tc.tile_pool(name="tp", bufs=2) as tp,
            tc.tile_pool(name="sp", bufs=2) as sp,
            tc.tile_pool(name="psc", bufs=6, space="PSUM") as psc,
            tc.tile_pool(name="ppx", bufs=1, space="PSUM") as ppx,
            tc.tile_pool(name="pph", bufs=1, space="PSUM") as pph,
        ):
            wx_sb = cp.tile([128, CIN, OC], f16)
            wh_sb = cp.tile([128, CHID, OH], f16)
            xht_sb = cp.tile([FEAT + 1, NPC], f16)
            rx_sb = cp.tile([FEAT + 1, OC], f16)
            rh_sb = cp.tile([FEAT + 1, OH], f16)
            hid_sb = cp.tile([128, NBLK, CHID], f32)
            hn_all = cp.tile([128, NBLK, CHID], f32)

            gc_t = [None] * NBLK
            a_t = [None] * NBLK

            def emit_loads(b):
                """per-block gather-feature + basis-matrix DMAs (SP queue)."""
                gc_t[b] = gcp.tile([64, CH_BLK, 2, FEAT], f8 if F8 else f16, name=f"gc{b}",
                                   tag="gc")
                nc.sync.dma_start(gc_t[b][:], gfv[:, b])
                a_t[b] = apool.tile([64, CH_BLK, 2, 128], f8 if F8 else f16,
                                    name=f"am{b}", tag="am")
                nc.sync.dma_start(a_t[b][:], amv[:, b])

            # gpsimd TensorTensor (GRU elementwise) runs from the ucode
            # standard library
            nc.gpsimd.load_library(library_config.standard)

            # block 0's front inputs first so PE starts ~10us earlier;
            # weights stream behind them and land before back(0) needs them
            emit_loads(0)
            nc.sync.dma_start(wx_sb[:], d_wx[:].rearrange("p (i o) -> p i o", o=OC))
            nc.sync.dma_start(wh_sb[:], d_wh[:].rearrange("p (i o) -> p i o", o=OH))
            nc.sync.dma_start(xht_sb[:], d_xht[:])
            nc.sync.dma_start(rx_sb[:], d_rx[:])
            nc.sync.dma_start(rh_sb[:], d_rh[:])
            nc.sync.dma_start(hid_sb[:], d_hid[:].rearrange("p (b c) -> p b c", c=CHID))

            def emit_front(b):
                """chunked scatter matmuls + 3-way rotated T drains.

                Chunk c holds two nodes' edge rows: node A on partitions
                0:64, node B on 64:128. Two 64-deep matmuls write T_A to
                PSUM cols 0:96 and T_B to 96:192 from a fully contiguous
                gathered-feature tile.
                """
                g = gc_t[b]
                a = a_t[b]
                t_t = tp.tile([128, BN, FEAT], f16)
                for j in range(CH_BLK // 2):
                    ps = psc.tile([128, 4 * FEAT], f32)
                    for h in range(4):
                        c, nd = 2 * j + h // 2, h % 2
                        nc.tensor.matmul(
                            ps[:, h * FEAT:(h + 1) * FEAT],
                            a[:, c, nd, :], g[:, c, nd, :],
                            start=True, stop=True,
                        )
                    dst = t_t[:, 4 * j:4 * j + 4, :]
                    # Pool/GPSIMD cannot access PSUM; alternate DVE/ACT
                    if j % 2 == 0:
                        nc.vector.tensor_copy(dst, ps[:])
                    else:
                        nc.scalar.copy(dst, ps[:])
                return t_t

            def emit_back(b, t_t):
                """dense conv matmuls + GRU combine into the output tile."""
                t_iv = t_t[:].rearrange("p n i -> p i n")
                px = ppx.tile([128, OC], f32)
                for i in range(CIN):
                    nc.tensor.matmul(
                        px[:], t_iv[:, i, :], wx_sb[:, i, :],
                        start=(i == 0), stop=False,
                    )
                ph = pph.tile([128, OH], f32)
                for i in range(CHID):
                    nc.tensor.matmul(
                        ph[:], t_iv[:, CIN + i, :], wh_sb[:, i, :],
                        start=(i == 0), stop=False,
                    )
                xht_blk = xht_sb[:, b * BN:(b + 1) * BN]
                nc.tensor.matmul(px[:], xht_blk, rx_sb[:], start=False, stop=True)
                nc.tensor.matmul(ph[:], xht_blk, rh_sb[:], start=False, stop=True)

                C = CHID
                ph_sb = sp.tile([128, OH], f32, tag="ph_sb")
                nc.scalar.copy(ph_sb[:], ph[:])
                a1 = sp.tile([128, C], f32, tag="a1")
                nc.vector.tensor_tensor(out=a1[:], in0=px[:, 0:C], in1=ph_sb[:, 0:C], op=alu.add)
                r = sp.tile([128, C], f32, tag="r")
                nc.scalar.activation(r[:], a1[:], act.Sigmoid)
                a2 = sp.tile([128, C], f32, tag="a2")
                nc.vector.tensor_tensor(out=a2[:], in0=px[:, C:2 * C], in1=ph_sb[:, C:2 * C], op=alu.add)
                z = sp.tile([128, C], f32, tag="z")
                nc.scalar.activation(z[:], a2[:], act.Sigmoid)
                t1 = sp.tile([128, C], f32, tag="t1")
                nc.gpsimd.tensor_tensor(out=t1[:], in0=r[:], in1=ph_sb[:, 0:C], op=alu.mult)
                t2 = sp.tile([128, C], f32, tag="t2")
                nc.vector.tensor_tensor(out=t2[:], in0=px[:, 2 * C:3 * C], in1=t1[:], op=alu.add)
                nn_ = sp.tile([128, C], f32, tag="nn")
                nc.scalar.activation(nn_[:], t2[:], act.Tanh)
                t3 = sp.tile([128, C], f32, tag="t3")
                nc.gpsimd.tensor_tensor(out=t3[:], in0=hid_sb[:, b, :], in1=nn_[:], op=alu.subtract)
                t4 = sp.tile([128, C], f32, tag="t4")
                nc.gpsimd.tensor_tensor(out=t4[:], in0=z[:], in1=t3[:], op=alu.mult)
                nc.gpsimd.tensor_tensor(out=hn_all[:, b, :], in0=nn_[:], in1=t4[:], op=alu.add)

            # software pipeline: back(b-1) is emitted after front(b) so the
            # PE stream never stalls on T copies at a block boundary
            pending = None
            for b in range(NBLK):
                if b + 1 < NBLK:
                    emit_loads(b + 1)
                t_t = emit_front(b)
                if pending is not None:
                    emit_back(pending[0], pending[1])
                pending = (b, t_t)
            emit_back(pending[0], pending[1])
            nc.sync.dma_start(
                d_out[:].rearrange("p (b c) -> p b c", c=CHID), hn_all[:]
            )

    nc.compile()
    return nc


def _plan_inputs(x, hidden, edge_index, edge_attr,
                 W_xr, root_xr, b_xr, W_hr, root_hr, b_hr,
                 W_xz, root_xz, b_xz, W_hz, root_hz, b_hz,
                 W_xn, root_xn, b_xn, W_hn=None, root_hn=None, b_hn=None):
    """Host-side sharding: group edges by destination core/node, build the
    per-core swizzled gather/basis arrays and packed weights."""
    src = np.asarray(edge_index[0], np.int64)
    dst = np.asarray(edge_index[1], np.int64)
    x = np.asarray(x, np.float32)
    hidden = np.asarray(hidden, np.float32)
    edge_attr = np.asarray(edge_attr, np.float32)

    deg = np.bincount(dst, minlength=N)
    if deg.max() > RPN:
        raise NotImplementedError(f"max degree {deg.max()} exceeds {RPN}")
    recip = 1.0 / np.maximum(deg, 1).astype(np.float32)

    # hat-basis weights per edge/dim: v[e, d, j] = relu(1 - |j - u_d|)
    u = edge_attr * (KS - 1)
    jj = np.arange(KS, dtype=np.float32)
    v = np.maximum(0.0, 1.0 - np.abs(jj[None, None, :] - u[:, :, None]))
    v1s = v[:, 0, :] * recip[dst][:, None]

    if F8:
        import ml_dtypes
        xdt = ml_dtypes.float8_e4m3fn
    else:
        xdt = np.float16
    xh = np.zeros((N, FEAT), xdt)
    xh[:, 0:CIN] = x.astype(xdt)
    xh[:, CIN:FEAT] = hidden.astype(xdt)
    wx = np.zeros((128, CIN, OC), np.float16)
    wx[:K, :, 0:CHID] = W_xr
    wx[:K, :, CHID:2 * CHID] = W_xz
    wx[:K, :, 2 * CHID:] = W_xn
    wh = np.zeros((128, CHID, OH), np.float16)
    wh[:K, :, 0:CHID] = W_hr
    wh[:K, :, CHID:] = W_hz
    rx = np.zeros((FEAT + 1, OC), np.float16)
    rx[0:CIN, 0:CHID] = root_xr
    rx[0:CIN, CHID:2 * CHID] = root_xz
    rx[0:CIN, 2 * CHID:] = root_xn
    rx[FEAT, :] = np.concatenate([b_xr, b_xz, b_xn]).astype(np.float16)
    rh = np.zeros((FEAT + 1, OH), np.float16)
    rh[CIN:FEAT, 0:CHID] = root_hr
    rh[CIN:FEAT, CHID:] = root_hz
    rh[FEAT, :] = np.concatenate([b_hr, b_hz]).astype(np.float16)

    in_maps = []
    for c in range(NCORES):
        lo = c * NPN
        sel = np.nonzero((dst >= lo) & (dst < lo + NPN))[0]
        order = sel[np.argsort(dst[sel], kind="stable")]
        sdst = dst[order]
        first = np.searchsorted(sdst, sdst, side="left")
        pos = np.arange(len(order)) - first
        rows = (sdst - lo) * RPN + pos  # in [0, NPC*RPN)

        esrc = np.zeros(NPC * RPN, np.int32)
        esrc[rows] = src[order]

        m = {
            "wx": np.ascontiguousarray(wx.reshape(128, CIN * OC)),
            "wh": np.ascontiguousarray(wh.reshape(128, CHID * OH)),
            "rx": rx, "rh": rh,
        }

        # A row = kron(v1*recip, v2, v3), fp32 accumulate, one rounding step
        if F8:
            import ml_dtypes
            adt = ml_dtypes.float8_e4m3fn
        else:
            adt = np.float16
        a_full = np.zeros((NPC * RPN, 128), adt)
        t25 = (v1s[order][:, :, None] * v[order][:, 1, None, :]).reshape(-1, 25)
        a_full[rows, 0:K] = (
            t25[:, :, None] * v[order][:, 2, None, :]
        ).reshape(-1, K).astype(adt)
        m["am"] = np.ascontiguousarray(
            a_full.reshape(NBLK * CH_BLK, 2, 64, 128).transpose(2, 0, 1, 3)
        ).reshape(64, NBLK * CH_BLK * 2 * 128)

        gath = xh[esrc]  # [NPC*RPN, FEAT]
        gf = gath.reshape(NBLK, CH_BLK, 2, 64, FEAT).transpose(3, 0, 1, 2, 4)
        m["gf"] = np.ascontiguousarray(gf).reshape(64, NBLK * CH_BLK * 2 * FEAT)

        xht = np.zeros((FEAT + 1, NPC), np.float16)
        xht[0:CIN, 0:NPN] = x[lo:lo + NPN].T
        xht[CIN:FEAT, 0:NPN] = hidden[lo:lo + NPN].T
        xht[FEAT, :] = 1.0
        m["xht"] = xht

        hid_pad = np.zeros((NPC, CHID), np.float32)
        hid_pad[0:NPN] = hidden[lo:lo + NPN]
        m["hid"] = np.ascontiguousarray(
            hid_pad.reshape(NBLK, 128, CHID).transpose(1, 0, 2)
        ).reshape(128, NBLK * CHID)

        in_maps.append(m)
    return in_maps


def kernel(**inputs):
    from concourse.bass_utils import run_bass_kernel_spmd

    if "nc" not in _cache:
        _cache["nc"] = _build_program()
    nc = _cache["nc"]

    in_maps = _plan_inputs(**inputs)
    kw = {}
    if os.environ.get("KERN_TRACE", ""):
        kw = dict(trace=True, trace_cores=[int(c) for c in
                  os.environ.get("KERN_TRACE_CORES", "0").split(",")])
        td = os.environ.get("KERN_TRACE_DIR", "")
        if td:
            kw["tmpdir"] = td
    res = run_bass_kernel_spmd(nc, in_maps, list(range(NCORES)), **kw)
    _cache["last"] = res

    out = np.empty((N, CHID), np.float32)
    for c in range(NCORES):
        blk = res.results[c]["out"].reshape(128, NBLK, CHID)
        full = blk.transpose(1, 0, 2).reshape(NPC, CHID)
        out[c * NPN:(c + 1) * NPN] = full[:NPN]
    return out
